# revision 18
# baseline (speedup 1.0000x reference)
"""Trainium2 Bass kernel: single-head causal self-attention.

Reference computation (per batch b, x: [S, D]):
    Q = x @ Wq ; K = x @ Wk ; V = x @ Wv
    S_sc = Q @ K^T / sqrt(D), causal masked
    out  = softmax(S_sc) @ V

Sharding: 8 cores, 4 batches -> core c handles batch b = c//2 and key
PARITY h = c%2: the 16 key tiles (128 rows each) of the batch are split
by tile parity, 8 tiles per core. Each core computes, for ALL 2048
queries, the partial softmax numerator n_c = sum_{k in parity} e^{s}*v
and denominator l_c; the host combines out = (n_0+n_1)/(l_0+l_1).
Uniform SPMD program; per-core behavior comes only from input data
(the parity-selected x columns and the global-k-index vector kg).

Key algebraic trick: scores = Q K^T = x (Wq Wk^T) x^T, so with
M = Wq @ Wk^T precomputed on host, A = x @ M replaces Q and the raw
x^T parity slice (already needed for the V projection) serves as the
stationary scores operand -- K is never projected on device.

All matmul operands are fp16 (same PE rate as f32r but half the DMA
and SBUF, letting V and A stay resident in SBUF; fp32 accumulation in
PSUM). Measured end-to-end rel err ~5e-4 vs the 2e-2 gate.

Softmax: no max-subtraction (scaled scores ~ N(0,1); exp fits fp16
range). Causal mask built on device from kg: only the diagonal k-tile
of each 256-wide q strip needs masking; earlier tiles are copied
straight out of the Exp activation.
"""

import sys

try:
    import concourse.bass as bass  # noqa: F401
except ImportError:
    sys.path.insert(0, "/opt/trn_rl_repo")

import numpy as np

import concourse.bass as bass  # noqa: F401
import concourse.tile as tile
from concourse import bacc, mybir
from concourse.bass_utils import run_bass_kernel_spmd

B, S, D = 4, 2048, 1024
P = 128
DT = D // P  # 8 d tiles
NKP = S // 2  # 1024 parity keys per core
KTP = NKP // P  # 8 parity k tiles
W = 256  # q-strip width
NSTRIP = S // W  # 8 strips
F32 = mybir.dt.float32
F16 = mybir.dt.float16
SCALE = 1.0 / np.sqrt(np.float32(D))  # 0.03125

_NC_CACHE = {}


def build_nc():
    nc = bacc.Bacc(None, target_bir_lowering=False)
    xkT_d = nc.dram_tensor("xkT", [D, NKP], F16, kind="ExternalInput")
    xqT_d = nc.dram_tensor("xqT", [D, S], F16, kind="ExternalInput")
    kg_d = nc.dram_tensor("kg", [NKP], F32, kind="ExternalInput")
    m_d = nc.dram_tensor("M", [D, D], F16, kind="ExternalInput")
    wv_d = nc.dram_tensor("Wv", [D, D], F16, kind="ExternalInput")
    nout_d = nc.dram_tensor("nout", [S, D], F16, kind="ExternalOutput")
    # lout is partition-major: lout[p*16 + t] = l[t*128 + p] (host transposes)
    lout_d = nc.dram_tensor("lout", [S], F32, kind="ExternalOutput")

    with tile.TileContext(nc) as tc:
        with (
            tc.tile_pool(name="persist", bufs=1) as persist,
            tc.tile_pool(name="misc", bufs=1) as misc,
        ):
            # Residents: x^T parity slice, A^T, V, weights
            xk = persist.tile([P, DT, NKP], F16, tag="xk")
            aT = persist.tile([P, DT, S], F16, tag="aT")
            vt = persist.tile([P, KTP, D], F16, tag="vt")
            m_w = persist.tile([P, DT, D], F16, tag="m_w")
            wv = persist.tile([P, DT, D], F16, tag="wv")

            # M arrives host-swizzled as m3[do*128+r, di*128+c] = M[di*128+r,
            # do*128+c]: one 256KB DMA block per do-column-block (2KB lines),
            # so aT group `do` gates on 256KB, not the whole 2MB. Blocks are
            # spread over all three DMA queues. m_w layout: [P, do, di*128+c];
            # the (do, di) stationary slice is m_w[:, do, di*128:(di+1)*128].
            xqT_t = xqT_d.rearrange("(a p) s -> p a s", p=P)
            CHUNKS = [128, 384, 512, 512, 512]  # graduated: PE starts early
            m_t = m_d.rearrange("(a p) o -> p a o", p=P)
            wv_t = wv_d.rearrange("(a p) o -> p a o", p=P)

            # ---------------- Phase 1: A^T = M^T x^T (resident) ----------------
            with (
                tc.tile_pool(name="xin", bufs=1) as xinp,
                tc.tile_pool(name="ps1", bufs=4, space="PSUM") as ps1,
            ):
                # DMA schedule (3 queues, orders matched to the compute
                # wavefront below): sync: xin0, xin1, M blocks 4-7;
                # scalar: M blocks 0-3, xin2; gpsimd: xin3, xin4, xk, wv.
                xins = []
                s0 = 0
                for ch, cw in enumerate(CHUNKS):
                    # one dedicated buffer per chunk: all DMAs in flight at once
                    xin = xinp.tile([P, DT, cw], F16, tag=f"xin{ch}", bufs=1)
                    xins.append((xin, s0, cw))
                    s0 += cw
                nc.sync.dma_start(xins[0][0], xqT_t[:, :, 0 : CHUNKS[0]])
                nc.sync.dma_start(
                    xins[1][0], xqT_t[:, :, xins[1][1] : xins[1][1] + CHUNKS[1]]
                )
                for do in range(4):
                    nc.scalar.dma_start(m_w[:, do, :], m_t[:, do, :])
                for do in range(4, DT):
                    nc.sync.dma_start(m_w[:, do, :], m_t[:, do, :])
                nc.scalar.dma_start(
                    xins[2][0], xqT_t[:, :, xins[2][1] : xins[2][1] + CHUNKS[2]]
                )
                kvec = misc.tile([P, KTP], F32, tag="kvec")
                nc.gpsimd.dma_start(kvec, kg_d.rearrange("(t p) -> p t", p=P))
                for ch in (3, 4):
                    nc.gpsimd.dma_start(
                        xins[ch][0], xqT_t[:, :, xins[ch][1] : xins[ch][1] + CHUNKS[ch]]
                    )
                xkT_t = xkT_d.rearrange("(a p) s -> p a s", p=P)
                nc.gpsimd.dma_start(xk, xkT_t)
                for di in range(DT):
                    nc.gpsimd.dma_start(wv[:, di, :], wv_t[:, di, :])

                # other constants
                ones = misc.tile([P, 2], F16, tag="ones")
                nc.vector.memset(ones, 1.0)
                qb_i = misc.tile([P, W], mybir.dt.int32, tag="qb_i")
                nc.gpsimd.iota(qb_i, pattern=[[1, W]], base=0, channel_multiplier=0)
                qbase = misc.tile([P, W], F32, tag="qbase")
                nc.vector.tensor_copy(qbase, qb_i)
                l_sb = misc.tile([P, S // P], F32, tag="l_sb")

                # wavefront: small chunks x low do-blocks first, tracking
                # the arrival order of M blocks and x chunks
                wave = (
                    [(c, do) for c in (0, 1) for do in range(4)]
                    + [(c, do) for c in (0, 1) for do in range(4, DT)]
                    + [(c, do) for c in (2, 3, 4) for do in range(DT)]
                )
                for ch, do in wave:
                    xin, s0, cw = xins[ch]
                    psfull = ps1.tile([P, 512], F32, tag="ps1", name="psfull")
                    ps = psfull[:, 0:cw]
                    for di in range(DT):
                        nc.tensor.matmul(
                            ps,
                            m_w[:, do, di * P : (di + 1) * P],
                            xin[:, di, :],
                            start=(di == 0),
                            stop=(di == DT - 1),
                        )
                    if do % 2 == 0:
                        nc.vector.tensor_copy(aT[:, do, s0 : s0 + cw], ps)
                    else:
                        nc.scalar.activation(
                            aT[:, do, s0 : s0 + cw],
                            ps,
                            mybir.ActivationFunctionType.Copy,
                        )

                # ---------------- Phase 2: V = x_k @ Wv (resident) ----------------
                # stationary: resident xk slices; moving: wv. No extra DMA.
                for kt in range(KTP):
                    for dh in range(2):
                        ps = ps1.tile([P, 512], F32, tag="ps1")
                        for di in range(DT):
                            nc.tensor.matmul(
                                ps,
                                xk[:, di, kt * P : (kt + 1) * P],
                                wv[:, di, dh * 512 : (dh + 1) * 512],
                                start=(di == 0),
                                stop=(di == DT - 1),
                            )
                        nc.vector.tensor_copy(vt[:, kt, dh * 512 : (dh + 1) * 512], ps)

            # ---------------- Phase 3: per-q-strip attention ----------------
            with (
                tc.tile_pool(name="strip", bufs=2) as strip,
                tc.tile_pool(name="sm", bufs=4) as sm,
                tc.tile_pool(name="outp", bufs=2) as outp,
                tc.tile_pool(name="ps2", bufs=2, space="PSUM") as ps2p,
                tc.tile_pool(name="psc", bufs=2, space="PSUM") as pscp,
                tc.tile_pool(name="psl", bufs=2, space="PSUM") as pslp,
            ):
                # largest strip first: the kernel ends on the cheapest strip,
                # shortening the post-PE drain
                for qs in reversed(range(NSTRIP)):
                    q0 = qs * W
                    # S^T strip -> exp -> (mask on diagonal tile) -> P^T
                    pT = strip.tile([P, KTP, W], F16, tag="pT")
                    for kt in range(qs + 1):
                        ps = ps2p.tile([P, W], F32, tag="ps2")
                        for di in range(DT):
                            nc.tensor.matmul(
                                ps,
                                xk[:, di, kt * P : (kt + 1) * P],
                                aT[:, di, q0 : q0 + W],
                                start=(di == 0),
                                stop=(di == DT - 1),
                            )
                        if kt < qs:
                            # fully below the diagonal: no mask needed
                            nc.scalar.activation(
                                pT[:, kt, :],
                                ps,
                                mybir.ActivationFunctionType.Exp,
                                scale=float(SCALE),
                            )
                        else:
                            et = sm.tile([P, W], F32, tag="et")
                            nc.scalar.activation(
                                et, ps, mybir.ActivationFunctionType.Exp,
                                scale=float(SCALE),
                            )
                            qgrid = sm.tile([P, W], F32, tag="qgrid")
                            nc.vector.tensor_scalar_add(qgrid, qbase, float(q0))
                            mt = sm.tile([P, W], F32, tag="mt")
                            nc.vector.tensor_scalar(
                                mt,
                                qgrid,
                                kvec[:, kt : kt + 1],
                                None,
                                op0=mybir.AluOpType.is_ge,
                            )
                            nc.vector.tensor_mul(pT[:, kt, :], et, mt)

                    # numerator = P^T.T @ V, denominator via ones column
                    ncq = W // P
                    cps = [
                        pscp.tile([P, D], F32, tag="psc", name=f"cps{i}")
                        for i in range(ncq)
                    ]
                    lps = [
                        pslp.tile([P, 2], F32, tag="psl", name=f"lps{i}")
                        for i in range(ncq)
                    ]
                    for kt in range(qs + 1):
                        for qt in range(ncq):
                            lhs = pT[:, kt, qt * P : (qt + 1) * P]
                            nc.tensor.matmul(
                                cps[qt][:, 0:512],
                                lhs,
                                vt[:, kt, 0:512],
                                start=(kt == 0),
                                stop=(kt == qs),
                            )
                            nc.tensor.matmul(
                                cps[qt][:, 512:1024],
                                lhs,
                                vt[:, kt, 512:1024],
                                start=(kt == 0),
                                stop=(kt == qs),
                            )
                            nc.tensor.matmul(
                                lps[qt],
                                lhs,
                                ones,
                                start=(kt == 0),
                                stop=(kt == qs),
                            )
                    for qt in range(ncq):
                        qi = 2 * qs + qt
                        nsb = outp.tile([P, D], F16, tag="nsb")
                        nc.vector.tensor_copy(nsb, cps[qt])
                        nc.sync.dma_start(nout_d[qi * P : (qi + 1) * P, :], nsb)
                        nc.vector.tensor_copy(l_sb[:, qi : qi + 1], lps[qt][:, 0:1])
                # partition-major: 64B contiguous per partition, hw queue
                nc.sync.dma_start(lout_d.rearrange("(p t) -> p t", p=P), l_sb)
    nc.compile()
    return nc


def _get_nc(key="f16"):
    if "nc" not in _NC_CACHE:
        _NC_CACHE["nc"] = build_nc()
    return _NC_CACHE["nc"]


def _ksel(h):
    """Local->global key indices for parity h: tiles h, 2+h, ..., 14+h."""
    tiles = np.arange(KTP) * 2 + h
    return (tiles[:, None] * P + np.arange(P)[None, :]).reshape(-1)


def make_in_maps(x, Wq, Wk, Wv):
    x = np.asarray(x, dtype=np.float32)
    Wq = np.asarray(Wq, dtype=np.float32)
    Wk = np.asarray(Wk, dtype=np.float32)
    Wv = np.asarray(Wv, dtype=np.float32)
    Mf = Wq @ Wk.T
    # swizzle so each 128-row block of the DRAM tensor carries one
    # do-column-block of M with all di slices: m3[do*128+r, di*128+c]
    # = M[di*128+r, do*128+c]
    M16 = np.ascontiguousarray(
        Mf.reshape(DT, P, DT, P).transpose(2, 1, 0, 3).reshape(D, D).astype(np.float16)
    )
    Wv16 = np.ascontiguousarray(Wv.astype(np.float16))
    in_maps = []
    for c in range(8):
        b, h = c // 2, c % 2
        ksel = _ksel(h)
        xbT16 = np.ascontiguousarray(x[b].T.astype(np.float16))
        in_maps.append(
            {
                "xkT": np.ascontiguousarray(xbT16[:, ksel]),
                "xqT": xbT16,
                "kg": ksel.astype(np.float32),
                "M": M16,
                "Wv": Wv16,
            }
        )
    return in_maps


def kernel(x, Wq, Wk, Wv, _trace=False, _nc_key="f16"):
    nc = _get_nc(_nc_key)
    in_maps = make_in_maps(x, Wq, Wk, Wv)
    res = run_bass_kernel_spmd(nc, in_maps, core_ids=list(range(8)), trace=_trace)
    out = np.empty((B, S, D), dtype=np.float32)
    for b in range(B):
        r0, r1 = res.results[2 * b], res.results[2 * b + 1]
        n = r0["nout"].astype(np.float32) + r1["nout"].astype(np.float32)
        # lout comes back partition-major: [p, t] -> global q = t*128 + p
        l = (r0["lout"] + r1["lout"]).reshape(P, S // P).T.reshape(-1)
        out[b] = n / l[:, None]
    if _trace:
        kernel.last_results = res
    return out


# revision 19
# speedup vs baseline: 1.0521x; 1.0521x over previous
"""Trainium2 Bass kernel: single-head causal self-attention.

Reference computation (per batch b, x: [S, D]):
    Q = x @ Wq ; K = x @ Wk ; V = x @ Wv
    S_sc = Q @ K^T / sqrt(D), causal masked
    out  = softmax(S_sc) @ V

Sharding: 8 cores, 4 batches -> core c handles batch b = c//2 and key
PARITY h = c%2: the 16 key tiles (128 rows each) of the batch are split
by tile parity, 8 tiles per core. Each core computes, for ALL 2048
queries, the partial softmax numerator n_c = sum_{k in parity} e^{s}*v
and denominator l_c; the host combines out = (n_0+n_1)/(l_0+l_1).
Uniform SPMD program; per-core behavior comes only from input data
(the parity-selected x columns and the global-k-index vector kg).

Key algebraic trick: scores = Q K^T = x (Wq Wk^T) x^T, so with
M = Wq @ Wk^T precomputed on host, A = x @ M replaces Q and the raw
x^T parity slice (already needed for the V projection) serves as the
stationary scores operand -- K is never projected on device.

All matmul operands are fp16 (same PE rate as f32r but half the DMA
and SBUF, letting V and A stay resident in SBUF; fp32 accumulation in
PSUM). Measured end-to-end rel err ~5e-4 vs the 2e-2 gate.

Softmax: no max-subtraction (scaled scores ~ N(0,1); exp fits fp16
range). Causal mask built on device from kg: only the diagonal k-tile
of each 256-wide q strip needs masking; earlier tiles are copied
straight out of the Exp activation.
"""

import sys

try:
    import concourse.bass as bass  # noqa: F401
except ImportError:
    sys.path.insert(0, "/opt/trn_rl_repo")

import numpy as np

import concourse.bass as bass  # noqa: F401
import concourse.tile as tile
from concourse import bacc, mybir
from concourse.bass_utils import run_bass_kernel_spmd

B, S, D = 4, 2048, 1024
P = 128
DT = D // P  # 8 d tiles
NKP = S // 2  # 1024 parity keys per core
KTP = NKP // P  # 8 parity k tiles
W = 256  # q-strip width
NSTRIP = S // W  # 8 strips
F32 = mybir.dt.float32
F16 = mybir.dt.float16
SCALE = 1.0 / np.sqrt(np.float32(D))  # 0.03125

_NC_CACHE = {}


def build_nc():
    nc = bacc.Bacc(None, target_bir_lowering=False)
    xkT_d = nc.dram_tensor("xkT", [D, NKP], F16, kind="ExternalInput")
    xqT_d = nc.dram_tensor("xqT", [D, S], F16, kind="ExternalInput")
    kg_d = nc.dram_tensor("kg", [NKP], F32, kind="ExternalInput")
    m_d = nc.dram_tensor("M", [D, D], F16, kind="ExternalInput")
    wv_d = nc.dram_tensor("Wv", [D, D], F16, kind="ExternalInput")
    nout_d = nc.dram_tensor("nout", [S, D], F16, kind="ExternalOutput")
    # lout is partition-major: lout[p*16 + t] = l[t*128 + p] (host transposes)
    lout_d = nc.dram_tensor("lout", [S], F32, kind="ExternalOutput")

    with tile.TileContext(nc) as tc:
        with (
            tc.tile_pool(name="persist", bufs=1) as persist,
            tc.tile_pool(name="misc", bufs=1) as misc,
        ):
            # Residents: x^T parity slice, A^T, V, weights
            xk = persist.tile([P, DT, NKP], F16, tag="xk")
            aT = persist.tile([P, DT, S], F16, tag="aT")
            vt = persist.tile([P, KTP, D], F16, tag="vt")
            m_w = persist.tile([P, DT, D], F16, tag="m_w")
            wv = persist.tile([P, DT, D], F16, tag="wv")

            # M arrives host-swizzled as m3[do*128+r, di*128+c] = M[di*128+r,
            # do*128+c]: one 256KB DMA block per do-column-block (2KB lines),
            # so aT group `do` gates on 256KB, not the whole 2MB. Blocks are
            # spread over all three DMA queues. m_w layout: [P, do, di*128+c];
            # the (do, di) stationary slice is m_w[:, do, di*128:(di+1)*128].
            xqT_t = xqT_d.rearrange("(a p) s -> p a s", p=P)
            CHUNKS = [128, 384, 512, 512, 512]  # graduated: PE starts early
            m_t = m_d.rearrange("(a p) o -> p a o", p=P)
            wv_t = wv_d.rearrange("(a p) o -> p a o", p=P)

            # ---------------- Phase 1: A^T = M^T x^T (resident) ----------------
            with (
                tc.tile_pool(name="xin", bufs=1) as xinp,
                tc.tile_pool(name="ps1", bufs=4, space="PSUM") as ps1,
            ):
                # DMA schedule (3 queues, orders matched to the compute
                # wavefront below): sync: xin0, xin1, M blocks 4-7;
                # scalar: M blocks 0-3, xin2; gpsimd: xin3, xin4, xk, wv.
                xins = []
                s0 = 0
                for ch, cw in enumerate(CHUNKS):
                    # one dedicated buffer per chunk: all DMAs in flight at once
                    xin = xinp.tile([P, DT, cw], F16, tag=f"xin{ch}", bufs=1)
                    xins.append((xin, s0, cw))
                    s0 += cw
                # measured: hw queues (sync/scalar) give ~260KB fast then
                # ~85GB/s; the gpsimd software queue starts ~12us in but
                # sustains ~250GB/s. So: hw queues carry only xin0 + M
                # (2.25MB); gpsimd carries all the bulk.
                nc.sync.dma_start(xins[0][0], xqT_t[:, :, 0 : CHUNKS[0]])
                for do in range(4):
                    nc.scalar.dma_start(m_w[:, do, :], m_t[:, do, :])
                for do in range(4, DT):
                    nc.sync.dma_start(m_w[:, do, :], m_t[:, do, :])
                for ch in (1, 2, 3, 4):
                    nc.gpsimd.dma_start(
                        xins[ch][0], xqT_t[:, :, xins[ch][1] : xins[ch][1] + CHUNKS[ch]]
                    )
                xkT_t = xkT_d.rearrange("(a p) s -> p a s", p=P)
                nc.gpsimd.dma_start(xk, xkT_t)
                for di in range(DT):
                    nc.gpsimd.dma_start(wv[:, di, :], wv_t[:, di, :])
                kvec = misc.tile([P, KTP], F32, tag="kvec")
                nc.scalar.dma_start(kvec, kg_d.rearrange("(t p) -> p t", p=P))

                # other constants
                ones = misc.tile([P, 2], F16, tag="ones")
                nc.vector.memset(ones, 1.0)
                qb_i = misc.tile([P, W], mybir.dt.int32, tag="qb_i")
                nc.gpsimd.iota(qb_i, pattern=[[1, W]], base=0, channel_multiplier=0)
                qbase = misc.tile([P, W], F32, tag="qbase")
                nc.vector.tensor_copy(qbase, qb_i)
                l_sb = misc.tile([P, S // P], F32, tag="l_sb")

                # wavefront: small chunks x low do-blocks first, tracking
                # the arrival order of M blocks and x chunks
                wave = (
                    [(c, do) for c in (0, 1) for do in range(4)]
                    + [(c, do) for c in (0, 1) for do in range(4, DT)]
                    + [(c, do) for c in (2, 3, 4) for do in range(DT)]
                )
                for ch, do in wave:
                    xin, s0, cw = xins[ch]
                    psfull = ps1.tile([P, 512], F32, tag="ps1", name="psfull")
                    ps = psfull[:, 0:cw]
                    for di in range(DT):
                        nc.tensor.matmul(
                            ps,
                            m_w[:, do, di * P : (di + 1) * P],
                            xin[:, di, :],
                            start=(di == 0),
                            stop=(di == DT - 1),
                        )
                    if do % 2 == 0:
                        nc.vector.tensor_copy(aT[:, do, s0 : s0 + cw], ps)
                    else:
                        nc.scalar.activation(
                            aT[:, do, s0 : s0 + cw],
                            ps,
                            mybir.ActivationFunctionType.Copy,
                        )

                # ---------------- Phase 2: V = x_k @ Wv (resident) ----------------
                # stationary: resident xk slices; moving: wv. No extra DMA.
                for kt in range(KTP):
                    for dh in range(2):
                        ps = ps1.tile([P, 512], F32, tag="ps1")
                        for di in range(DT):
                            nc.tensor.matmul(
                                ps,
                                xk[:, di, kt * P : (kt + 1) * P],
                                wv[:, di, dh * 512 : (dh + 1) * 512],
                                start=(di == 0),
                                stop=(di == DT - 1),
                            )
                        nc.vector.tensor_copy(vt[:, kt, dh * 512 : (dh + 1) * 512], ps)

            # ---------------- Phase 3: per-q-strip attention ----------------
            with (
                tc.tile_pool(name="strip", bufs=2) as strip,
                tc.tile_pool(name="sm", bufs=4) as sm,
                tc.tile_pool(name="outp", bufs=2) as outp,
                tc.tile_pool(name="ps2", bufs=2, space="PSUM") as ps2p,
                tc.tile_pool(name="psc", bufs=2, space="PSUM") as pscp,
                tc.tile_pool(name="psl", bufs=2, space="PSUM") as pslp,
            ):
                # largest strip first: the kernel ends on the cheapest strip,
                # shortening the post-PE drain
                for qs in reversed(range(NSTRIP)):
                    q0 = qs * W
                    # S^T strip -> exp -> (mask on diagonal tile) -> P^T
                    pT = strip.tile([P, KTP, W], F16, tag="pT")
                    for kt in range(qs + 1):
                        ps = ps2p.tile([P, W], F32, tag="ps2")
                        for di in range(DT):
                            nc.tensor.matmul(
                                ps,
                                xk[:, di, kt * P : (kt + 1) * P],
                                aT[:, di, q0 : q0 + W],
                                start=(di == 0),
                                stop=(di == DT - 1),
                            )
                        if kt < qs:
                            # fully below the diagonal: no mask needed
                            nc.scalar.activation(
                                pT[:, kt, :],
                                ps,
                                mybir.ActivationFunctionType.Exp,
                                scale=float(SCALE),
                            )
                        else:
                            et = sm.tile([P, W], F32, tag="et")
                            nc.scalar.activation(
                                et, ps, mybir.ActivationFunctionType.Exp,
                                scale=float(SCALE),
                            )
                            qgrid = sm.tile([P, W], F32, tag="qgrid")
                            nc.vector.tensor_scalar_add(qgrid, qbase, float(q0))
                            mt = sm.tile([P, W], F32, tag="mt")
                            nc.vector.tensor_scalar(
                                mt,
                                qgrid,
                                kvec[:, kt : kt + 1],
                                None,
                                op0=mybir.AluOpType.is_ge,
                            )
                            nc.vector.tensor_mul(pT[:, kt, :], et, mt)

                    # numerator = P^T.T @ V, denominator via ones column
                    ncq = W // P
                    cps = [
                        pscp.tile([P, D], F32, tag="psc", name=f"cps{i}")
                        for i in range(ncq)
                    ]
                    lps = [
                        pslp.tile([P, 2], F32, tag="psl", name=f"lps{i}")
                        for i in range(ncq)
                    ]
                    for kt in range(qs + 1):
                        for qt in range(ncq):
                            lhs = pT[:, kt, qt * P : (qt + 1) * P]
                            nc.tensor.matmul(
                                cps[qt][:, 0:512],
                                lhs,
                                vt[:, kt, 0:512],
                                start=(kt == 0),
                                stop=(kt == qs),
                            )
                            nc.tensor.matmul(
                                cps[qt][:, 512:1024],
                                lhs,
                                vt[:, kt, 512:1024],
                                start=(kt == 0),
                                stop=(kt == qs),
                            )
                            nc.tensor.matmul(
                                lps[qt],
                                lhs,
                                ones,
                                start=(kt == 0),
                                stop=(kt == qs),
                            )
                    for qt in range(ncq):
                        qi = 2 * qs + qt
                        nsb = outp.tile([P, D], F16, tag="nsb")
                        nc.vector.tensor_copy(nsb, cps[qt])
                        nc.sync.dma_start(nout_d[qi * P : (qi + 1) * P, :], nsb)
                        nc.vector.tensor_copy(l_sb[:, qi : qi + 1], lps[qt][:, 0:1])
                # partition-major: 64B contiguous per partition, hw queue
                nc.sync.dma_start(lout_d.rearrange("(p t) -> p t", p=P), l_sb)
    nc.compile()
    return nc


def _get_nc(key="f16"):
    if "nc" not in _NC_CACHE:
        _NC_CACHE["nc"] = build_nc()
    return _NC_CACHE["nc"]


def _ksel(h):
    """Local->global key indices for parity h: tiles h, 2+h, ..., 14+h."""
    tiles = np.arange(KTP) * 2 + h
    return (tiles[:, None] * P + np.arange(P)[None, :]).reshape(-1)


def make_in_maps(x, Wq, Wk, Wv):
    x = np.asarray(x, dtype=np.float32)
    Wq = np.asarray(Wq, dtype=np.float32)
    Wk = np.asarray(Wk, dtype=np.float32)
    Wv = np.asarray(Wv, dtype=np.float32)
    Mf = Wq @ Wk.T
    # swizzle so each 128-row block of the DRAM tensor carries one
    # do-column-block of M with all di slices: m3[do*128+r, di*128+c]
    # = M[di*128+r, do*128+c]
    M16 = np.ascontiguousarray(
        Mf.reshape(DT, P, DT, P).transpose(2, 1, 0, 3).reshape(D, D).astype(np.float16)
    )
    Wv16 = np.ascontiguousarray(Wv.astype(np.float16))
    in_maps = []
    for c in range(8):
        b, h = c // 2, c % 2
        ksel = _ksel(h)
        xbT16 = np.ascontiguousarray(x[b].T.astype(np.float16))
        in_maps.append(
            {
                "xkT": np.ascontiguousarray(xbT16[:, ksel]),
                "xqT": xbT16,
                "kg": ksel.astype(np.float32),
                "M": M16,
                "Wv": Wv16,
            }
        )
    return in_maps


def kernel(x, Wq, Wk, Wv, _trace=False, _nc_key="f16"):
    nc = _get_nc(_nc_key)
    in_maps = make_in_maps(x, Wq, Wk, Wv)
    res = run_bass_kernel_spmd(nc, in_maps, core_ids=list(range(8)), trace=_trace)
    out = np.empty((B, S, D), dtype=np.float32)
    for b in range(B):
        r0, r1 = res.results[2 * b], res.results[2 * b + 1]
        n = r0["nout"].astype(np.float32) + r1["nout"].astype(np.float32)
        # lout comes back partition-major: [p, t] -> global q = t*128 + p
        l = (r0["lout"] + r1["lout"]).reshape(P, S // P).T.reshape(-1)
        out[b] = n / l[:, None]
    if _trace:
        kernel.last_results = res
    return out


# revision 21
# speedup vs baseline: 1.0545x; 1.0023x over previous
"""Trainium2 Bass kernel: single-head causal self-attention.

Reference computation (per batch b, x: [S, D]):
    Q = x @ Wq ; K = x @ Wk ; V = x @ Wv
    S_sc = Q @ K^T / sqrt(D), causal masked
    out  = softmax(S_sc) @ V

Sharding: 8 cores, 4 batches -> core c handles batch b = c//2 and key
PARITY h = c%2: the 16 key tiles (128 rows each) of the batch are split
by tile parity, 8 tiles per core. Each core computes, for ALL 2048
queries, the partial softmax numerator n_c = sum_{k in parity} e^{s}*v
and denominator l_c; the host combines out = (n_0+n_1)/(l_0+l_1).
Uniform SPMD program; per-core behavior comes only from input data
(the parity-selected x columns and the global-k-index vector kg).

Key algebraic trick: scores = Q K^T = x (Wq Wk^T) x^T, so with
M = Wq @ Wk^T precomputed on host, A = x @ M replaces Q and the raw
x^T parity slice (already needed for the V projection) serves as the
stationary scores operand -- K is never projected on device.

All matmul operands are fp16 (same PE rate as f32r but half the DMA
and SBUF, letting V and A stay resident in SBUF; fp32 accumulation in
PSUM). Measured end-to-end rel err ~5e-4 vs the 2e-2 gate.

Softmax: no max-subtraction (scaled scores ~ N(0,1); exp fits fp16
range). Causal mask built on device from kg: only the diagonal k-tile
of each 256-wide q strip needs masking; earlier tiles are copied
straight out of the Exp activation.
"""

import sys

try:
    import concourse.bass as bass  # noqa: F401
except ImportError:
    sys.path.insert(0, "/opt/trn_rl_repo")

import numpy as np

import concourse.bass as bass  # noqa: F401
import concourse.tile as tile
from concourse import bacc, mybir
from concourse.bass_utils import run_bass_kernel_spmd

B, S, D = 4, 2048, 1024
P = 128
DT = D // P  # 8 d tiles
NKP = S // 2  # 1024 parity keys per core
KTP = NKP // P  # 8 parity k tiles
W = 256  # q-strip width
NSTRIP = S // W  # 8 strips
F32 = mybir.dt.float32
F16 = mybir.dt.float16
SCALE = 1.0 / np.sqrt(np.float32(D))  # 0.03125

_NC_CACHE = {}


def build_nc():
    nc = bacc.Bacc(None, target_bir_lowering=False)
    xkT_d = nc.dram_tensor("xkT", [D, NKP], F16, kind="ExternalInput")
    xqT_d = nc.dram_tensor("xqT", [D, S], F16, kind="ExternalInput")
    kg_d = nc.dram_tensor("kg", [NKP], F32, kind="ExternalInput")
    m_d = nc.dram_tensor("M", [D, D], F16, kind="ExternalInput")
    wv_d = nc.dram_tensor("Wv", [D, D], F16, kind="ExternalInput")
    nout_d = nc.dram_tensor("nout", [S, D], F16, kind="ExternalOutput")
    # lout is partition-major: lout[p*16 + t] = l[t*128 + p] (host transposes)
    lout_d = nc.dram_tensor("lout", [S], F32, kind="ExternalOutput")

    with tile.TileContext(nc) as tc:
        with (
            tc.tile_pool(name="persist", bufs=1) as persist,
            tc.tile_pool(name="misc", bufs=1) as misc,
        ):
            # Residents: x^T parity slice, A^T, V, weights
            xk = persist.tile([P, DT, NKP], F16, tag="xk")
            aT = persist.tile([P, DT, S], F16, tag="aT")
            vt = persist.tile([P, KTP, D], F16, tag="vt")
            m_w = persist.tile([P, DT, D], F16, tag="m_w")
            wv = persist.tile([P, DT, D], F16, tag="wv")

            # M arrives host-swizzled as m3[do*128+r, di*128+c] = M[di*128+r,
            # do*128+c]: one 256KB DMA block per do-column-block (2KB lines),
            # so aT group `do` gates on 256KB, not the whole 2MB. Blocks are
            # spread over all three DMA queues. m_w layout: [P, do, di*128+c];
            # the (do, di) stationary slice is m_w[:, do, di*128:(di+1)*128].
            xqT_t = xqT_d.rearrange("(a p) s -> p a s", p=P)
            CHUNKS = [128, 384, 512, 512, 512]  # graduated: PE starts early
            m_t = m_d.rearrange("(a p) o -> p a o", p=P)
            wv_t = wv_d.rearrange("(a p) o -> p a o", p=P)

            # ---------------- Phase 1: A^T = M^T x^T (resident) ----------------
            with (
                tc.tile_pool(name="xin", bufs=1) as xinp,
                tc.tile_pool(name="ps1", bufs=4, space="PSUM") as ps1,
            ):
                # DMA schedule (3 queues, orders matched to the compute
                # wavefront below): sync: xin0, xin1, M blocks 4-7;
                # scalar: M blocks 0-3, xin2; gpsimd: xin3, xin4, xk, wv.
                xins = []
                s0 = 0
                for ch, cw in enumerate(CHUNKS):
                    # one dedicated buffer per chunk: all DMAs in flight at once
                    xin = xinp.tile([P, DT, cw], F16, tag=f"xin{ch}", bufs=1)
                    xins.append((xin, s0, cw))
                    s0 += cw
                # measured: hw queues (sync/scalar) give ~260KB fast then
                # ~85GB/s; the gpsimd software queue starts ~12us in but
                # sustains ~250GB/s. So: hw queues carry only xin0 + M
                # (2.25MB); gpsimd carries all the bulk.
                nc.sync.dma_start(xins[0][0], xqT_t[:, :, 0 : CHUNKS[0]])
                for do in range(3):
                    nc.scalar.dma_start(m_w[:, do, :], m_t[:, do, :])
                for do in range(3, 6):
                    nc.sync.dma_start(m_w[:, do, :], m_t[:, do, :])
                for do in range(6, DT):
                    nc.gpsimd.dma_start(m_w[:, do, :], m_t[:, do, :])
                for ch in (1, 2, 3, 4):
                    nc.gpsimd.dma_start(
                        xins[ch][0], xqT_t[:, :, xins[ch][1] : xins[ch][1] + CHUNKS[ch]]
                    )
                xkT_t = xkT_d.rearrange("(a p) s -> p a s", p=P)
                nc.gpsimd.dma_start(xk, xkT_t)
                for di in range(DT):
                    nc.gpsimd.dma_start(wv[:, di, :], wv_t[:, di, :])
                kvec = misc.tile([P, KTP], F32, tag="kvec")
                nc.scalar.dma_start(kvec, kg_d.rearrange("(t p) -> p t", p=P))

                # other constants
                ones = misc.tile([P, 2], F16, tag="ones")
                nc.vector.memset(ones, 1.0)
                qb_i = misc.tile([P, W], mybir.dt.int32, tag="qb_i")
                nc.gpsimd.iota(qb_i, pattern=[[1, W]], base=0, channel_multiplier=0)
                qbase = misc.tile([P, W], F32, tag="qbase")
                nc.vector.tensor_copy(qbase, qb_i)
                l_sb = misc.tile([P, S // P], F32, tag="l_sb")

                # wavefront: small chunks x low do-blocks first, tracking
                # the arrival order of M blocks and x chunks
                wave = (
                    [(c, do) for c in (0, 1) for do in range(4)]
                    + [(c, do) for c in (0, 1) for do in range(4, DT)]
                    + [(c, do) for c in (2, 3, 4) for do in range(DT)]
                )
                for ch, do in wave:
                    xin, s0, cw = xins[ch]
                    psfull = ps1.tile([P, 512], F32, tag="ps1", name="psfull")
                    ps = psfull[:, 0:cw]
                    for di in range(DT):
                        nc.tensor.matmul(
                            ps,
                            m_w[:, do, di * P : (di + 1) * P],
                            xin[:, di, :],
                            start=(di == 0),
                            stop=(di == DT - 1),
                        )
                    if do % 2 == 0:
                        nc.vector.tensor_copy(aT[:, do, s0 : s0 + cw], ps)
                    else:
                        nc.scalar.activation(
                            aT[:, do, s0 : s0 + cw],
                            ps,
                            mybir.ActivationFunctionType.Copy,
                        )

                # ---------------- Phase 2: V = x_k @ Wv (resident) ----------------
                # stationary: resident xk slices; moving: wv. No extra DMA.
                for kt in range(KTP):
                    for dh in range(2):
                        ps = ps1.tile([P, 512], F32, tag="ps1")
                        for di in range(DT):
                            nc.tensor.matmul(
                                ps,
                                xk[:, di, kt * P : (kt + 1) * P],
                                wv[:, di, dh * 512 : (dh + 1) * 512],
                                start=(di == 0),
                                stop=(di == DT - 1),
                            )
                        nc.vector.tensor_copy(vt[:, kt, dh * 512 : (dh + 1) * 512], ps)

            # ---------------- Phase 3: per-q-strip attention ----------------
            with (
                tc.tile_pool(name="strip", bufs=2) as strip,
                tc.tile_pool(name="sm", bufs=4) as sm,
                tc.tile_pool(name="outp", bufs=2) as outp,
                tc.tile_pool(name="ps2", bufs=2, space="PSUM") as ps2p,
                tc.tile_pool(name="psc", bufs=2, space="PSUM") as pscp,
                tc.tile_pool(name="psl", bufs=2, space="PSUM") as pslp,
            ):
                # largest strip first: the kernel ends on the cheapest strip,
                # shortening the post-PE drain
                for qs in reversed(range(NSTRIP)):
                    q0 = qs * W
                    # S^T strip -> exp -> (mask on diagonal tile) -> P^T
                    pT = strip.tile([P, KTP, W], F16, tag="pT")
                    for kt in range(qs + 1):
                        ps = ps2p.tile([P, W], F32, tag="ps2")
                        for di in range(DT):
                            nc.tensor.matmul(
                                ps,
                                xk[:, di, kt * P : (kt + 1) * P],
                                aT[:, di, q0 : q0 + W],
                                start=(di == 0),
                                stop=(di == DT - 1),
                            )
                        if kt < qs:
                            # fully below the diagonal: no mask needed
                            nc.scalar.activation(
                                pT[:, kt, :],
                                ps,
                                mybir.ActivationFunctionType.Exp,
                                scale=float(SCALE),
                            )
                        else:
                            et = sm.tile([P, W], F32, tag="et")
                            nc.scalar.activation(
                                et, ps, mybir.ActivationFunctionType.Exp,
                                scale=float(SCALE),
                            )
                            qgrid = sm.tile([P, W], F32, tag="qgrid")
                            nc.vector.tensor_scalar_add(qgrid, qbase, float(q0))
                            mt = sm.tile([P, W], F32, tag="mt")
                            nc.vector.tensor_scalar(
                                mt,
                                qgrid,
                                kvec[:, kt : kt + 1],
                                None,
                                op0=mybir.AluOpType.is_ge,
                            )
                            nc.vector.tensor_mul(pT[:, kt, :], et, mt)

                    # numerator = P^T.T @ V, denominator via ones column
                    ncq = W // P
                    cps = [
                        pscp.tile([P, D], F32, tag="psc", name=f"cps{i}")
                        for i in range(ncq)
                    ]
                    lps = [
                        pslp.tile([P, 2], F32, tag="psl", name=f"lps{i}")
                        for i in range(ncq)
                    ]
                    for kt in range(qs + 1):
                        for qt in range(ncq):
                            lhs = pT[:, kt, qt * P : (qt + 1) * P]
                            nc.tensor.matmul(
                                cps[qt][:, 0:512],
                                lhs,
                                vt[:, kt, 0:512],
                                start=(kt == 0),
                                stop=(kt == qs),
                            )
                            nc.tensor.matmul(
                                cps[qt][:, 512:1024],
                                lhs,
                                vt[:, kt, 512:1024],
                                start=(kt == 0),
                                stop=(kt == qs),
                            )
                            nc.tensor.matmul(
                                lps[qt],
                                lhs,
                                ones,
                                start=(kt == 0),
                                stop=(kt == qs),
                            )
                    for qt in range(ncq):
                        qi = 2 * qs + qt
                        nsb = outp.tile([P, D], F16, tag="nsb")
                        nc.vector.tensor_copy(nsb, cps[qt])
                        nc.gpsimd.dma_start(nout_d[qi * P : (qi + 1) * P, :], nsb)
                        nc.vector.tensor_copy(l_sb[:, qi : qi + 1], lps[qt][:, 0:1])
                # partition-major: 64B contiguous per partition
                nc.gpsimd.dma_start(lout_d.rearrange("(p t) -> p t", p=P), l_sb)
    nc.compile()
    return nc


def _get_nc(key="f16"):
    if "nc" not in _NC_CACHE:
        _NC_CACHE["nc"] = build_nc()
    return _NC_CACHE["nc"]


def _ksel(h):
    """Local->global key indices for parity h: tiles h, 2+h, ..., 14+h."""
    tiles = np.arange(KTP) * 2 + h
    return (tiles[:, None] * P + np.arange(P)[None, :]).reshape(-1)


def make_in_maps(x, Wq, Wk, Wv):
    x = np.asarray(x, dtype=np.float32)
    Wq = np.asarray(Wq, dtype=np.float32)
    Wk = np.asarray(Wk, dtype=np.float32)
    Wv = np.asarray(Wv, dtype=np.float32)
    Mf = Wq @ Wk.T
    # swizzle so each 128-row block of the DRAM tensor carries one
    # do-column-block of M with all di slices: m3[do*128+r, di*128+c]
    # = M[di*128+r, do*128+c]
    M16 = np.ascontiguousarray(
        Mf.reshape(DT, P, DT, P).transpose(2, 1, 0, 3).reshape(D, D).astype(np.float16)
    )
    Wv16 = np.ascontiguousarray(Wv.astype(np.float16))
    in_maps = []
    for c in range(8):
        b, h = c // 2, c % 2
        ksel = _ksel(h)
        xbT16 = np.ascontiguousarray(x[b].T.astype(np.float16))
        in_maps.append(
            {
                "xkT": np.ascontiguousarray(xbT16[:, ksel]),
                "xqT": xbT16,
                "kg": ksel.astype(np.float32),
                "M": M16,
                "Wv": Wv16,
            }
        )
    return in_maps


def kernel(x, Wq, Wk, Wv, _trace=False, _nc_key="f16"):
    nc = _get_nc(_nc_key)
    in_maps = make_in_maps(x, Wq, Wk, Wv)
    res = run_bass_kernel_spmd(nc, in_maps, core_ids=list(range(8)), trace=_trace)
    out = np.empty((B, S, D), dtype=np.float32)
    for b in range(B):
        r0, r1 = res.results[2 * b], res.results[2 * b + 1]
        n = r0["nout"].astype(np.float32) + r1["nout"].astype(np.float32)
        # lout comes back partition-major: [p, t] -> global q = t*128 + p
        l = (r0["lout"] + r1["lout"]).reshape(P, S // P).T.reshape(-1)
        out[b] = n / l[:, None]
    if _trace:
        kernel.last_results = res
    return out
